# Initial kernel scaffold
#
"""Multi-head attention (B=2, L=S=2048, D=1024, H=16) on 8 Trainium2 cores.

Sharding: core c -> batch b = c // 4, head group g = c % 4 (4 heads per core).
W_Q/K/V column-sharded (256 cols per core), W_O row-sharded (256 rows per core);
the 4 partial outputs per batch are summed on the host (plus bias terms).

Per-core pipeline (all big tensors kept transposed so no on-device transposes):
  projections: QT = 0.125*(x Wq + bq)^T, KT = (x Wk + bk)^T (feature-major
    [256, L]); Vaug = [V_h | ones] per head (seq-major, fp16), V bias folded
    out on the host (softmax rows sum to 1 => + bv @ Wo + bo once).
  attention, per (l-tile 512, s-tile 128): S^T = KT^T QT (row-packed pairs of
    heads, K=64); E = exp(S^T) * maskT (ACT exp from PSUM, 0/1 fp16 mask
    multiply on DVE at 2x); T_h += Vaug_h^T E accumulates BOTH the head
    output AND its softmax row-sums in one full-array matmul (ones columns
    act as the reducer; even heads get [V|1] -> av in rows 0:64, odd heads
    [1|V] -> av in rows 64:128 so every result lands on the lanes the
    output-projection layout needs). Per l-tile: reciprocal_approx_fast on
    the sum half, DMA lane-swap to the av half's partitions, multiply into
    outT (fp16).
  out-projection: out_partial = outT^T Wo_rows (K=128, accumulate over the
    two 128-row groups).

All matmul operands fp16 (1 cyc/row, no packing restrictions); PSUM fp32.
One shared pool scope - PSUM budget 8 banks = scores 2x2 + T_h 4x1; the
projection and output-projection matmuls borrow the same slots, so phases
overlap without pool-boundary barriers.
"""
from contextlib import ExitStack

import numpy as np

import concourse.bass as bass
import concourse.mybir as mybir
import concourse.tile as tile
from concourse import bacc
from concourse.bass_utils import run_bass_kernel_spmd

F16 = mybir.dt.float16
F32 = mybir.dt.float32

D = 1024          # d_model
H = 16            # heads
DK = 64           # head dim
B, L = 2, 2048
NCORES = 8
HPC = 4           # heads per core
FPC = HPC * DK    # features per core = 256
KD = D // 128     # 8 contraction subtiles for projections
LT, LTW = 4, 512  # l tiles
ST, STW = 16, 128  # s tiles
Ident = mybir.ActivationFunctionType.Identity
Exp = mybir.ActivationFunctionType.Exp

_CACHED_NC = None


def _build():
    nc = bacc.Bacc("TRN2", target_bir_lowering=False, debug=False,
                   num_devices=NCORES)
    xT = nc.declare_dram_parameter("xT", [128, KD, L], F16, isOutput=False)
    wq = nc.declare_dram_parameter("wq", [128, KD, FPC], F16, isOutput=False)
    wk = nc.declare_dram_parameter("wk", [128, KD, FPC], F16, isOutput=False)
    wv = nc.declare_dram_parameter("wv", [128, KD, FPC], F16, isOutput=False)
    wo = nc.declare_dram_parameter("wo", [128, 2, D], F16, isOutput=False)
    bq = nc.declare_dram_parameter("bq", [128, 2], F32, isOutput=False)
    bk = nc.declare_dram_parameter("bk", [128, 2], F32, isOutput=False)
    maskT = nc.declare_dram_parameter("maskT", [ST, LT, 128, LTW], F16,
                                      isOutput=False)
    out = nc.declare_dram_parameter("out", [128, ST, D], F16, isOutput=True)

    with tile.TileContext(nc) as tc, ExitStack() as ctx:
        pool = ctx.enter_context(tc.tile_pool(name="pers", bufs=1))
        mpool = ctx.enter_context(tc.tile_pool(name="mpool", bufs=4))
        epool = ctx.enter_context(tc.tile_pool(name="epool", bufs=4))
        rbpool = ctx.enter_context(tc.tile_pool(name="rbpool", bufs=4))
        opool = ctx.enter_context(tc.tile_pool(name="opool", bufs=3))
        scp = ctx.enter_context(tc.tile_pool(name="scp", bufs=2, space="PSUM"))
        tp = ctx.enter_context(tc.tile_pool(name="tp", bufs=1, space="PSUM"))

        xt = pool.tile([128, KD, L], F16)
        wq_sb = pool.tile([128, KD, FPC], F16)
        wk_sb = pool.tile([128, KD, FPC], F16)
        wv_sb = pool.tile([128, KD, FPC], F16)
        wo_sb = pool.tile([128, 2, D], F16)
        bq_sb = pool.tile([128, 2], F32)
        bk_sb = pool.tile([128, 2], F32)
        # DMA issue order follows the dependency order of the first
        # matmuls: KT needs wk + xt chunk k; V needs wv + xt chunk k.
        nc.sync.dma_start(out=wk_sb[:], in_=wk[:])
        nc.sync.dma_start(out=xt[:, 0, :], in_=xT[:, 0, :])
        nc.sync.dma_start(out=wv_sb[:], in_=wv[:])
        for kd in range(1, KD):
            nc.sync.dma_start(out=xt[:, kd, :], in_=xT[:, kd, :])
        nc.sync.dma_start(out=wq_sb[:], in_=wq[:])
        nc.sync.dma_start(out=bk_sb[:], in_=bk[:])
        nc.sync.dma_start(out=bq_sb[:], in_=bq[:])
        nc.sync.dma_start(out=wo_sb[:], in_=wo[:])

        QT = pool.tile([128, 2, L], F16)   # [feat(2x128), l]: Q^T * 0.125
        KT = pool.tile([128, 2, L], F16)
        # Vaug[:, st, h]: even h -> [V_h | 1], odd h -> [1 | V_h]
        Vaug = pool.tile([128, ST, HPC, 128], F16)
        nc.gpsimd.memset(Vaug[:], 1.0)
        outTs = [pool.tile([128, 2, LTW], F16, name=f"outT{i}")
                 for i in range(LT)]

        # ---- emission interleaves projections into the attention loop so
        # ---- the PE absorbs them while ACT (exp) is the bottleneck.
        def emit_kt_chunk(c):
            lsl = slice(c * LTW, (c + 1) * LTW)
            ps = scp.tile([128, 2, LTW], F32, tag="sc", name=f"pk{c}")
            for ft in range(2):
                fsl = slice(ft * 128, (ft + 1) * 128)
                for kd in range(KD):
                    nc.tensor.matmul(ps[:, ft, :], wk_sb[:, kd, fsl],
                                     xt[:, kd, lsl],
                                     start=(kd == 0), stop=(kd == KD - 1))
                nc.vector.scalar_tensor_tensor(
                    KT[:, ft, lsl], ps[:, ft, :], 1.0,
                    bk_sb[:, ft:ft + 1].to_broadcast((128, LTW)),
                    mybir.AluOpType.mult, mybir.AluOpType.add)

        def emit_v_chunk(c):
            for st in range(4 * c, 4 * c + 4):
                ssl = slice(st * STW, (st + 1) * STW)
                psv = tp.tile([128, LTW], F32, tag=f"T{st % 4}", name=f"psv{st}")
                for kd in range(KD):
                    nc.tensor.matmul(psv[:, :FPC], xt[:, kd, ssl],
                                     wv_sb[:, kd, :],
                                     start=(kd == 0), stop=(kd == KD - 1))
                for h in range(HPC):
                    off = 0 if h % 2 == 0 else 64
                    nc.vector.tensor_copy(Vaug[:, st, h, off:off + 64],
                                          psv[:, DK * h:DK * (h + 1)])

        def emit_qt(lt):
            lsl = slice(lt * LTW, (lt + 1) * LTW)
            psq = scp.tile([128, 2, LTW], F32, tag="sc", name=f"pq{lt}")
            for ft in range(2):
                fsl = slice(ft * 128, (ft + 1) * 128)
                for kd in range(KD):
                    nc.tensor.matmul(psq[:, ft, :], wq_sb[:, kd, fsl],
                                     xt[:, kd, lsl],
                                     start=(kd == 0), stop=(kd == KD - 1))
                nc.scalar.activation(QT[:, ft, lsl], psq[:, ft, :], Ident,
                                     bias=bq_sb[:, ft:ft + 1], scale=0.125)

        for c in range(4):
            emit_kt_chunk(c)
            emit_v_chunk(c)
        emit_qt(0)

        for lt in range(LT):
            lsl = slice(lt * LTW, (lt + 1) * LTW)
            if lt > 0:
                emit_qt(lt)
            Ts = [tp.tile([128, LTW], F32, tag=f"T{h}", name=f"T{h}_{lt}")
                  for h in range(HPC)]
            for st in range(ST):
                ssl = slice(st * STW, (st + 1) * STW)
                mk = mpool.tile([128, LTW], F16)
                nc.sync.dma_start(out=mk[:], in_=maskT[st, lt])
                Es = []
                for pair in range(2):
                    sc = scp.tile([128, 2, LTW], F32, tag="sc")
                    for i in range(2):
                        nc.tensor.matmul(
                            sc[:, i, :],
                            KT[64 * i:64 * (i + 1), pair, ssl],
                            QT[64 * i:64 * (i + 1), pair, lsl],
                            start=True, stop=True)
                    E = epool.tile([128, 2, LTW], F16, name=f"E{pair}")
                    nc.scalar.activation(E[:], sc[:], Exp)
                    nc.vector.tensor_mul(
                        E[:], E[:],
                        mk[:, None, :].to_broadcast((128, 2, LTW)))
                    Es.append(E)
                # all four aug matmuls back-to-back: one weight-swap drain
                # boundary per s-tile instead of one per pair
                for pair in range(2):
                    for i in range(2):
                        h = 2 * pair + i
                        nc.tensor.matmul(Ts[h][:], Vaug[:, st, h, :],
                                         Es[pair][:, i, :],
                                         start=(st == 0), stop=(st == ST - 1))
            for h in range(HPC):
                # reciprocal_approx_fast only works at partition base 0, so
                # route the row sums through lanes 0:64 in both parities.
                pair, i = divmod(h, 2)
                av_sl = slice(64 * i, 64 * (i + 1))        # av lanes
                rs_sl = slice(64 * (1 - i), 64 * (2 - i))  # row-sum lanes
                rb = rbpool.tile([128, LTW], F32)
                if i == 0:   # av 0:64, sums 64:128 -> move sums down first
                    nc.vector.tensor_copy(rb[64:128, :], Ts[h][rs_sl, :])
                    nc.gpsimd.dma_start(out=rb[0:64, :], in_=rb[64:128, :])
                    nc.vector.reciprocal_approx_fast(out=rb[0:64, :],
                                                     in_=rb[0:64, :])
                else:        # sums 0:64 -> recip at base 0, then move up
                    nc.vector.reciprocal_approx_fast(out=rb[0:64, :],
                                                     in_=Ts[h][rs_sl, :])
                    nc.gpsimd.dma_start(out=rb[64:128, :], in_=rb[0:64, :])
                nc.vector.tensor_mul(outTs[lt][av_sl, pair, :],
                                     Ts[h][av_sl, :], rb[av_sl, :])

        # ---------------- output projection ----------------
        for lt8 in range(ST):
            ps3 = scp.tile([128, 2, LTW], F32, tag="sc", name=f"ps3_{lt8}")
            for nf in range(2):
                nsl = slice(nf * 512, (nf + 1) * 512)
                for pair in range(2):
                    nc.tensor.matmul(
                        ps3[:, nf, :],
                        outTs[lt8 // 4][:, pair,
                                        (lt8 % 4) * 128:(lt8 % 4 + 1) * 128],
                        wo_sb[:, pair, nsl],
                        start=(pair == 0), stop=(pair == 1))
            ob = opool.tile([128, D], F16)
            if lt8 % 2 == 0:
                nc.scalar.copy(ob[:], ps3[:])
            else:
                nc.vector.tensor_copy(ob[:], ps3[:])
            nc.gpsimd.dma_start(out=out[:, lt8, :], in_=ob[:])

    nc.compile()
    return nc


def _get_nc():
    global _CACHED_NC
    if _CACHED_NC is None:
        _CACHED_NC = _build()
    return _CACHED_NC


def _prep_core_inputs(c, x, mask, Wq, bq, Wk, bk, Wv, Wo):
    b, g = divmod(c, 4)
    cs = slice(g * FPC, (g + 1) * FPC)

    xT = np.ascontiguousarray(
        x[b].T.reshape(KD, 128, L).transpose(1, 0, 2)).astype(np.float16)
    wq_c = np.ascontiguousarray(
        Wq[:, cs].reshape(KD, 128, FPC).transpose(1, 0, 2)).astype(np.float16)
    wk_c = np.ascontiguousarray(
        Wk[:, cs].reshape(KD, 128, FPC).transpose(1, 0, 2)).astype(np.float16)
    wv_c = np.ascontiguousarray(
        Wv[:, cs].reshape(KD, 128, FPC).transpose(1, 0, 2)).astype(np.float16)
    wo_c = np.ascontiguousarray(
        Wo[cs, :].reshape(2, 128, D).transpose(1, 0, 2)).astype(np.float16)
    bq_c = np.ascontiguousarray(
        (bq[cs] * 0.125).reshape(2, 128).T).astype(np.float32)
    bk_c = np.ascontiguousarray(bk[cs].reshape(2, 128).T).astype(np.float32)
    mT = mask[b].astype(np.float16).T  # [S, L]
    maskT = np.ascontiguousarray(
        mT.reshape(ST, 128, LT, LTW).transpose(0, 2, 1, 3))
    return {"xT": xT, "wq": wq_c, "wk": wk_c, "wv": wv_c, "wo": wo_c,
            "bq": bq_c, "bk": bk_c, "maskT": maskT}


def kernel(x, mask, Wq, bq, Wk, bk, Wv, bv, Wo, bo):
    x = np.asarray(x, np.float32)
    mask = np.asarray(mask)
    Wq, bq = np.asarray(Wq, np.float32), np.asarray(bq, np.float32)
    Wk, bk = np.asarray(Wk, np.float32), np.asarray(bk, np.float32)
    Wv, bv = np.asarray(Wv, np.float32), np.asarray(bv, np.float32)
    Wo, bo = np.asarray(Wo, np.float32), np.asarray(bo, np.float32)

    nc = _get_nc()
    in_maps = [_prep_core_inputs(c, x, mask, Wq, bq, Wk, bk, Wv, Wo)
               for c in range(NCORES)]
    res = run_bass_kernel_spmd(nc, in_maps, list(range(NCORES)))

    const_vec = (bv @ Wo + bo).astype(np.float32)  # A rows sum to 1
    outs = []
    for b in range(B):
        acc = np.zeros((L, D), np.float32)
        for g in range(4):
            part = res.results[4 * b + g]["out"]  # [128, 16, 1024] fp16
            acc += part.transpose(1, 0, 2).reshape(L, D).astype(np.float32)
        acc += const_vec
        outs.append(acc)
    return np.stack(outs)



# revision 1
# speedup vs baseline: 1.1332x; 1.1332x over previous
"""Multi-head attention (B=2, L=S=2048, D=1024, H=16) on 8 Trainium2 cores.

Sharding: core c -> batch b = c // 4, head group g = c % 4 (4 heads per core).
W_Q/K/V column-sharded (256 cols per core), W_O row-sharded (256 rows per core);
the 4 partial outputs per batch are summed on the host (plus bias terms).

Per-core pipeline (all big tensors kept transposed so no on-device transposes):
  projections: QT = 0.125*(x Wq + bq)^T, KT = (x Wk + bk)^T (feature-major
    [256, L]); Vaug = [V_h | ones] per head (seq-major, fp16), V bias folded
    out on the host (softmax rows sum to 1 => + bv @ Wo + bo once).
  attention, per (l-tile 512, s-tile 128): S^T = KT^T QT (row-packed pairs of
    heads, K=64); E = exp(S^T) * maskT (ACT exp from PSUM, 0/1 fp16 mask
    multiply on DVE at 2x); T_h += Vaug_h^T E accumulates BOTH the head
    output AND its softmax row-sums in one full-array matmul (ones columns
    act as the reducer; even heads get [V|1] -> av in rows 0:64, odd heads
    [1|V] -> av in rows 64:128 so every result lands on the lanes the
    output-projection layout needs). Per l-tile: reciprocal_approx_fast on
    the sum half, DMA lane-swap to the av half's partitions, multiply into
    outT (fp16).
  out-projection: out_partial = outT^T Wo_rows (K=128, accumulate over the
    two 128-row groups).

All matmul operands fp16 (1 cyc/row, no packing restrictions); PSUM fp32.
One shared pool scope - PSUM budget 8 banks = scores 2x2 + T_h 4x1; the
projection and output-projection matmuls borrow the same slots, so phases
overlap without pool-boundary barriers.
"""
from contextlib import ExitStack

import numpy as np

import concourse.bass as bass
import concourse.mybir as mybir
import concourse.tile as tile
from concourse import bacc
from concourse.bass_utils import run_bass_kernel_spmd

F16 = mybir.dt.float16
F32 = mybir.dt.float32

D = 1024          # d_model
H = 16            # heads
DK = 64           # head dim
B, L = 2, 2048
NCORES = 8
HPC = 4           # heads per core
FPC = HPC * DK    # features per core = 256
KD = D // 128     # 8 contraction subtiles for projections
LT, LTW = 4, 512  # l tiles
ST, STW = 16, 128  # s tiles
Ident = mybir.ActivationFunctionType.Identity
Exp = mybir.ActivationFunctionType.Exp

_CACHED_NC = None


def _build():
    nc = bacc.Bacc("TRN2", target_bir_lowering=False, debug=False,
                   num_devices=NCORES)
    xT = nc.declare_dram_parameter("xT", [128, KD, L], F16, isOutput=False)
    wq = nc.declare_dram_parameter("wq", [128, KD, FPC], F16, isOutput=False)
    wk = nc.declare_dram_parameter("wk", [128, KD, FPC], F16, isOutput=False)
    wv = nc.declare_dram_parameter("wv", [128, KD, FPC], F16, isOutput=False)
    wo = nc.declare_dram_parameter("wo", [128, 2, D], F16, isOutput=False)
    bq = nc.declare_dram_parameter("bq", [128, 2], F32, isOutput=False)
    bk = nc.declare_dram_parameter("bk", [128, 2], F32, isOutput=False)
    maskT = nc.declare_dram_parameter("maskT", [ST, LT, 128, LTW], F16,
                                      isOutput=False)
    out = nc.declare_dram_parameter("out", [128, ST, D], F16, isOutput=True)

    with tile.TileContext(nc) as tc, ExitStack() as ctx:
        pool = ctx.enter_context(tc.tile_pool(name="pers", bufs=1))
        mpool = ctx.enter_context(tc.tile_pool(name="mpool", bufs=4))
        epool = ctx.enter_context(tc.tile_pool(name="epool", bufs=4))
        rbpool = ctx.enter_context(tc.tile_pool(name="rbpool", bufs=4))
        opool = ctx.enter_context(tc.tile_pool(name="opool", bufs=3))
        scp = ctx.enter_context(tc.tile_pool(name="scp", bufs=2, space="PSUM"))
        tp = ctx.enter_context(tc.tile_pool(name="tp", bufs=1, space="PSUM"))

        xt = pool.tile([128, KD, L], F16)
        wq_sb = pool.tile([128, KD, FPC], F16)
        wk_sb = pool.tile([128, KD, FPC], F16)
        wv_sb = pool.tile([128, KD, FPC], F16)
        wo_sb = pool.tile([128, 2, D], F16)
        bq_sb = pool.tile([128, 2], F32)
        bk_sb = pool.tile([128, 2], F32)
        # DMA issue order follows the dependency order of the first
        # matmuls: KT needs wk + xt chunk k; V needs wv + xt chunk k.
        nc.sync.dma_start(out=wk_sb[:], in_=wk[:])
        nc.sync.dma_start(out=xt[:, 0, :], in_=xT[:, 0, :])
        nc.sync.dma_start(out=wv_sb[:], in_=wv[:])
        for kd in range(1, KD):
            nc.sync.dma_start(out=xt[:, kd, :], in_=xT[:, kd, :])
        nc.sync.dma_start(out=wq_sb[:], in_=wq[:])
        nc.sync.dma_start(out=bk_sb[:], in_=bk[:])
        nc.sync.dma_start(out=bq_sb[:], in_=bq[:])
        nc.sync.dma_start(out=wo_sb[:], in_=wo[:])

        QT = pool.tile([128, 2, L], F16)   # [feat(2x128), l]: Q^T * 0.125
        KT = pool.tile([128, 2, L], F16)
        # Vaug[:, st, h]: even h -> [V_h | 1], odd h -> [1 | V_h]
        Vaug = pool.tile([128, ST, HPC, 128], F16)
        nc.gpsimd.memset(Vaug[:], 1.0)
        outTs = [pool.tile([128, 2, LTW], F16, name=f"outT{i}")
                 for i in range(LT)]

        # ---- emission interleaves projections into the attention loop so
        # ---- the PE absorbs them while ACT (exp) is the bottleneck.
        def emit_kt_chunk(c):
            lsl = slice(c * LTW, (c + 1) * LTW)
            ps = scp.tile([128, 2, LTW], F32, tag="sc", name=f"pk{c}")
            for ft in range(2):
                fsl = slice(ft * 128, (ft + 1) * 128)
                for kd in range(KD):
                    nc.tensor.matmul(ps[:, ft, :], wk_sb[:, kd, fsl],
                                     xt[:, kd, lsl],
                                     start=(kd == 0), stop=(kd == KD - 1))
                nc.vector.scalar_tensor_tensor(
                    KT[:, ft, lsl], ps[:, ft, :], 1.0,
                    bk_sb[:, ft:ft + 1].to_broadcast((128, LTW)),
                    mybir.AluOpType.mult, mybir.AluOpType.add)

        def emit_v_chunk(c):
            for st in range(4 * c, 4 * c + 4):
                ssl = slice(st * STW, (st + 1) * STW)
                psv = tp.tile([128, LTW], F32, tag=f"T{st % 4}", name=f"psv{st}")
                for kd in range(KD):
                    nc.tensor.matmul(psv[:, :FPC], xt[:, kd, ssl],
                                     wv_sb[:, kd, :],
                                     start=(kd == 0), stop=(kd == KD - 1))
                for h in range(HPC):
                    off = 0 if h % 2 == 0 else 64
                    nc.vector.tensor_copy(Vaug[:, st, h, off:off + 64],
                                          psv[:, DK * h:DK * (h + 1)])

        def emit_qt(lt):
            lsl = slice(lt * LTW, (lt + 1) * LTW)
            psq = scp.tile([128, 2, LTW], F32, tag="sc", name=f"pq{lt}")
            for ft in range(2):
                fsl = slice(ft * 128, (ft + 1) * 128)
                for kd in range(KD):
                    nc.tensor.matmul(psq[:, ft, :], wq_sb[:, kd, fsl],
                                     xt[:, kd, lsl],
                                     start=(kd == 0), stop=(kd == KD - 1))
                nc.scalar.activation(QT[:, ft, lsl], psq[:, ft, :], Ident,
                                     bias=bq_sb[:, ft:ft + 1], scale=0.125)

        for c in range(4):
            emit_kt_chunk(c)
            emit_v_chunk(c)
        emit_qt(0)

        for lt in range(LT):
            lsl = slice(lt * LTW, (lt + 1) * LTW)
            if lt > 0:
                emit_qt(lt)
            Ts = [tp.tile([128, LTW], F32, tag=f"T{h}", name=f"T{h}_{lt}")
                  for h in range(HPC)]
            for st in range(ST):
                ssl = slice(st * STW, (st + 1) * STW)
                mk = mpool.tile([128, LTW], F16)
                nc.sync.dma_start(out=mk[:], in_=maskT[st, lt])
                Es = []
                for pair in range(2):
                    sc = scp.tile([128, 2, LTW], F32, tag="sc")
                    for i in range(2):
                        nc.tensor.matmul(
                            sc[:, i, :],
                            KT[64 * i:64 * (i + 1), pair, ssl],
                            QT[64 * i:64 * (i + 1), pair, lsl],
                            start=True, stop=True)
                    E = epool.tile([128, 2, LTW], F16, name=f"E{pair}")
                    nc.scalar.activation(E[:], sc[:], Exp)
                    nc.vector.tensor_mul(
                        E[:], E[:],
                        mk[:, None, :].to_broadcast((128, 2, LTW)))
                    Es.append(E)
                # all four aug matmuls back-to-back: one weight-swap drain
                # boundary per s-tile instead of one per pair
                for pair in range(2):
                    for i in range(2):
                        h = 2 * pair + i
                        nc.tensor.matmul(Ts[h][:], Vaug[:, st, h, :],
                                         Es[pair][:, i, :],
                                         start=(st == 0), stop=(st == ST - 1))
            for h in range(HPC):
                # reciprocal_approx_fast only works at partition base 0, so
                # route the row sums through lanes 0:64 in both parities.
                pair, i = divmod(h, 2)
                av_sl = slice(64 * i, 64 * (i + 1))        # av lanes
                rs_sl = slice(64 * (1 - i), 64 * (2 - i))  # row-sum lanes
                rb = rbpool.tile([128, LTW], F32)
                if i == 0:   # av 0:64, sums 64:128 -> move sums down first
                    nc.vector.tensor_copy(rb[64:128, :], Ts[h][rs_sl, :])
                    nc.gpsimd.dma_start(out=rb[0:64, :], in_=rb[64:128, :])
                    nc.vector.reciprocal_approx_fast(out=rb[0:64, :],
                                                     in_=rb[0:64, :])
                else:        # sums 0:64 -> recip at base 0, then move up
                    nc.vector.reciprocal_approx_fast(out=rb[0:64, :],
                                                     in_=Ts[h][rs_sl, :])
                    nc.gpsimd.dma_start(out=rb[64:128, :], in_=rb[0:64, :])
                nc.vector.tensor_mul(outTs[lt][av_sl, pair, :],
                                     Ts[h][av_sl, :], rb[av_sl, :])

        # ---------------- output projection ----------------
        for lt8 in range(ST):
            ps3 = scp.tile([128, 2, LTW], F32, tag="sc", name=f"ps3_{lt8}")
            for nf in range(2):
                nsl = slice(nf * 512, (nf + 1) * 512)
                for pair in range(2):
                    nc.tensor.matmul(
                        ps3[:, nf, :],
                        outTs[lt8 // 4][:, pair,
                                        (lt8 % 4) * 128:(lt8 % 4 + 1) * 128],
                        wo_sb[:, pair, nsl],
                        start=(pair == 0), stop=(pair == 1))
            ob = opool.tile([128, D], F16)
            if lt8 % 2 == 0:
                nc.scalar.copy(ob[:], ps3[:])
            else:
                nc.vector.tensor_copy(ob[:], ps3[:])
            nc.gpsimd.dma_start(out=out[:, lt8, :], in_=ob[:])

    nc.compile()
    return nc


def _get_nc():
    global _CACHED_NC
    if _CACHED_NC is None:
        _CACHED_NC = _build()
    return _CACHED_NC


def _prep_core_inputs(c, x, mask, Wq, bq, Wk, bk, Wv, Wo):
    b, g = divmod(c, 4)
    cs = slice(g * FPC, (g + 1) * FPC)

    xT = np.ascontiguousarray(
        x[b].T.reshape(KD, 128, L).transpose(1, 0, 2)).astype(np.float16)
    wq_c = np.ascontiguousarray(
        Wq[:, cs].reshape(KD, 128, FPC).transpose(1, 0, 2)).astype(np.float16)
    wk_c = np.ascontiguousarray(
        Wk[:, cs].reshape(KD, 128, FPC).transpose(1, 0, 2)).astype(np.float16)
    wv_c = np.ascontiguousarray(
        Wv[:, cs].reshape(KD, 128, FPC).transpose(1, 0, 2)).astype(np.float16)
    wo_c = np.ascontiguousarray(
        Wo[cs, :].reshape(2, 128, D).transpose(1, 0, 2)).astype(np.float16)
    bq_c = np.ascontiguousarray(
        (bq[cs] * 0.125).reshape(2, 128).T).astype(np.float32)
    bk_c = np.ascontiguousarray(bk[cs].reshape(2, 128).T).astype(np.float32)
    mT = mask[b].astype(np.float16).T  # [S, L]
    maskT = np.ascontiguousarray(
        mT.reshape(ST, 128, LT, LTW).transpose(0, 2, 1, 3))
    return {"xT": xT, "wq": wq_c, "wk": wk_c, "wv": wv_c, "wo": wo_c,
            "bq": bq_c, "bk": bk_c, "maskT": maskT}


def kernel(x, mask, Wq, bq, Wk, bk, Wv, bv, Wo, bo):
    x = np.asarray(x, np.float32)
    mask = np.asarray(mask)
    Wq, bq = np.asarray(Wq, np.float32), np.asarray(bq, np.float32)
    Wk, bk = np.asarray(Wk, np.float32), np.asarray(bk, np.float32)
    Wv, bv = np.asarray(Wv, np.float32), np.asarray(bv, np.float32)
    Wo, bo = np.asarray(Wo, np.float32), np.asarray(bo, np.float32)

    nc = _get_nc()
    in_maps = [_prep_core_inputs(c, x, mask, Wq, bq, Wk, bk, Wv, Wo)
               for c in range(NCORES)]
    res = run_bass_kernel_spmd(nc, in_maps, list(range(NCORES)))

    const_vec = (bv @ Wo + bo).astype(np.float32)  # A rows sum to 1
    outs = []
    for b in range(B):
        acc = np.zeros((L, D), np.float32)
        for g in range(4):
            part = res.results[4 * b + g]["out"]  # [128, 16, 1024] fp16
            acc += part.transpose(1, 0, 2).reshape(L, D).astype(np.float32)
        acc += const_vec
        outs.append(acc)
    return np.stack(outs)

